# revision 6
# baseline (speedup 1.0000x reference)
"""GRU-D decoder kernel for Trainium2 (8 NeuronCores, data-parallel over batch).

Math (mask == ones everywhere, which the reference hardcodes):
  x_hat = C (constant), d = dt broadcast, gamma_x unused.
  gamma[t,b,j] = exp(-relu(dt[t,b] * colsum(Wgh)[j] + bgh[j]))   (precomputed host-side)
  per step: hd = gamma_t * h
            z = sigmoid(hd @ Wz_h + Az0);  r = sigmoid(hd @ Wr_h + Ar0)
            htl = tanh((r*hd) @ Wh_h + Ah0)
            h = hd + z*(htl - hd)
  out[t] = h_t @ Wlin + blin
  where A?0 = C @ W?_x + colsum(W?_m) + b?  (time-constant, precomputed host-side).

Device design (v4):
  - Transposed world: H folded onto 128 partitions (4 chunks of 128), batch=64
    on the free axis.  Weight-stationary (form 2) gate matmuls, N=64 streams.
  - Gate weights quantized to fp8e4 * 256 (stationary; fast weight load), moving
    operand hd stays bf16; the 1/256 is folded into the activation scale.
  - State update with gamma folded in:
      hd' = u + v,  u = e1 - z*e1 (pre-tanh),  v = e2*tanh(...)
      e1 = gamma'*hd (GPSIMD, step start),  e2 = gamma'*z.
  - Software-pipelined r-gate: since hd' = u + v and matmul is linear, the
    NEXT step's r-gate matmuls run as two accumulation passes -- u@Wr during
    the tanh window (PE would otherwise idle -> HAM re-throttles the clock)
    and v@Wr immediately after v lands.  The z-gate runs as a single pass on
    hd' during the next step's sigmoid(r) window.  This keeps tensor-engine
    duty high enough to hold the K=8/8 clock and takes the update tail off
    the matmul critical path.
  - z and r share one PSUM bank [128, 512]; one ident-matmul streams the
    (pre-scaled) gate constants to start both accumulation groups.
  - h_t = hd + z*(htl-hd) recomputed on GPSIMD (slack-tolerant) into a big
    h-history tile; output projection (form 2, bf16 Wlin) runs as a rolling
    4-step-batched matmul split around the candidate matmuls; +blin is fused
    into an ACT Identity-with-bias PSUM evacuation.  Output DMA'd as bf16.
"""

import numpy as np
import ml_dtypes

T, B, H, O = 100, 512, 512, 512
NCORES = 8
BL = B // NCORES  # 64
KC = 4            # contraction chunks of 128
JT = 4            # output j-tiles of 128
FR = JT * BL      # 256
HB = FR // 2      # 128 (half of the free dim)
GCH = 10          # gamma chunk (steps per DMA)
PJ = 4            # projection flush period (steps)
WSCALE = 256.0    # fp8 gate-weight scale (undone in the activation)

_BUILD_CACHE = {}


def _build_program():
    if "nc" in _BUILD_CACHE:
        return _BUILD_CACHE["nc"]

    import concourse.tile as tile
    import concourse.mybir as mybir
    from concourse import bacc
    from contextlib import ExitStack

    f32 = mybir.dt.float32
    bf16 = mybir.dt.bfloat16
    f8 = mybir.dt.float8e4
    AF = mybir.ActivationFunctionType

    nc = bacc.Bacc("TRN2", target_bir_lowering=False, debug=False,
                   num_devices=NCORES)

    gam_d = nc.dram_tensor("gam", [128, T, FR], bf16, kind="ExternalInput")
    wg8_d = nc.dram_tensor("wg8", [128, 3 * KC * JT * 128], f8, kind="ExternalInput")
    wlin_d = nc.dram_tensor("wlin", [128, KC * JT * 128], bf16, kind="ExternalInput")
    a0zr_d = nc.dram_tensor("a0zr", [128, 2 * FR], bf16, kind="ExternalInput")
    a0h_d = nc.dram_tensor("a0h", [128, FR], bf16, kind="ExternalInput")
    ident_d = nc.dram_tensor("ident", [128, 128], f8, kind="ExternalInput")
    blin_d = nc.dram_tensor("blin", [128, JT], f32, kind="ExternalInput")
    out_d = nc.dram_tensor("out", [JT, 128, T, BL], bf16, kind="ExternalOutput")

    with tile.TileContext(nc) as tc, ExitStack() as ctx:
        constp = ctx.enter_context(tc.tile_pool(name="const", bufs=1))
        bigp = ctx.enter_context(tc.tile_pool(name="big", bufs=1))
        statep = ctx.enter_context(tc.tile_pool(name="state", bufs=2))
        workp = ctx.enter_context(tc.tile_pool(name="work", bufs=2))
        stagep = ctx.enter_context(tc.tile_pool(name="stage", bufs=3))
        pzrp = ctx.enter_context(tc.tile_pool(name="pzr", bufs=2, space="PSUM"))
        phtp = ctx.enter_context(tc.tile_pool(name="pht", bufs=2, space="PSUM"))
        ppjp = ctx.enter_context(tc.tile_pool(name="ppj", bufs=1, space="PSUM"))

        wg8 = constp.tile([128, 3 * KC * JT * 128], f8)
        nc.sync.dma_start(wg8[:], wg8_d[:])
        wlin = constp.tile([128, KC * JT * 128], bf16)
        nc.sync.dma_start(wlin[:], wlin_d[:])
        a0zr = constp.tile([128, 2 * FR], bf16)
        nc.sync.dma_start(a0zr[:], a0zr_d[:])
        a0h = constp.tile([128, FR], bf16)
        nc.sync.dma_start(a0h[:], a0h_d[:])
        ident = constp.tile([128, 128], f8)
        nc.sync.dma_start(ident[:], ident_d[:])
        blin = constp.tile([128, JT], f32)
        nc.sync.dma_start(blin[:], blin_d[:])

        # gamma history + h history as big 3D tiles; chunked DMA into slices
        gam = bigp.tile([128, T, FR], bf16)
        hs = bigp.tile([128, T, FR], bf16)

        def gam_fetch(c):
            t0 = c * GCH
            t1 = min(t0 + GCH, T)
            if t0 < T:
                nc.sync.dma_start(gam[:, t0:t1, :], gam_d[:, t0:t1, :])

        gam_fetch(0)
        gam_fetch(1)

        def wg_blk(g, kc, jo):
            i = ((g * KC + kc) * JT + jo) * 128
            return wg8[:, i:i + 128]

        def wl_blk(kc, m):
            i = (kc * JT + m) * 128
            return wlin[:, i:i + 128]

        hdb = statep.tile([128, FR], bf16, tag="hdb")
        nc.vector.memset(hdb[:], 0.0)

        def proj_mms(m, base, kcs, pj):
            for kc in kcs:
                nc.tensor.matmul(
                    pj[:, 0:PJ * BL],
                    wl_blk(kc, m),
                    hs[:, base:base + PJ, kc * BL:(kc + 1) * BL],
                    start=(kc == 0), stop=(kc == KC - 1),
                )

        def proj_evac(m, base, pj):
            stg = stagep.tile([128, PJ * BL], bf16, tag="stg")
            nc.scalar.activation(stg[:], pj[:], AF.Identity, bias=blin[:, m:m + 1])
            nc.sync.dma_start(out_d[m][:, base:base + PJ, :], stg[:])

        def init_gates(zr, ht):
            nc.tensor.matmul(zr[:], ident[:], a0zr[:], start=True, stop=False)
            nc.tensor.matmul(ht[:], ident[:], a0h[:], start=True, stop=False)

        def rpass(zr, src, kcs):
            # r-gate accumulation pass over contraction chunks kcs of src
            for kc in kcs:
                for jo in range(JT):
                    nc.tensor.matmul(
                        zr[:, FR + jo * BL:FR + (jo + 1) * BL],
                        wg_blk(1, kc, jo), src[:, kc * BL:(kc + 1) * BL],
                        start=False, stop=False,
                    )

        # step-0 gate PSUM groups (hd_0 = 0, so constants only for r)
        zr = pzrp.tile([128, 2 * FR], f32, tag="zr")
        ht = phtp.tile([128, FR], f32, tag="ht")
        init_gates(zr, ht)

        for t in range(T):
            c, o = divmod(t, GCH)
            if o == 0 and t > 0:
                gam_fetch(c + 1)
            last = t + 1 >= T

            if not last:
                gn = gam[:, t + 1, :]
                # e1 = gamma' * hd   (GPSIMD, ready at step start, off-path)
                e1 = workp.tile([128, FR], bf16, tag="e1")
                nc.gpsimd.tensor_mul(e1[:], gn, hdb[:])

            # ---- sigmoid(r) -> rh in halves (r PSUM completed last iter)
            rb = workp.tile([128, FR], bf16, tag="rb")
            rh = workp.tile([128, FR], bf16, tag="rh")
            for h0 in (0, HB):
                nc.scalar.activation(rb[:, h0:h0 + HB], zr[:, FR + h0:FR + h0 + HB],
                                     AF.Sigmoid, scale=1.0 / WSCALE)
                nc.vector.tensor_mul(rh[:, h0:h0 + HB], rb[:, h0:h0 + HB],
                                     hdb[:, h0:h0 + HB])

            # ---- z-gate single pass on hd (runs during sigmoid(r) window)
            for kc in range(KC):
                for jo in range(JT):
                    nc.tensor.matmul(
                        zr[:, jo * BL:(jo + 1) * BL],
                        wg_blk(0, kc, jo), hdb[:, kc * BL:(kc + 1) * BL],
                        start=False, stop=(jo == JT - 1 and kc == KC - 1),
                    )

            # ---- rolling projection (PE filler) for steps [pbase, pbase+PJ)
            pj = pbase = None
            if t >= PJ:
                pbase = (t // PJ - 1) * PJ
                pj = ppjp.tile([128, PJ * BL], f32, tag=f"pj{t % PJ}")
                proj_mms(t % PJ, pbase, (0, 1), pj)

            # ---- candidate MMs (k-outer: kc chunk waits only on its rh half)
            for kc in range(KC):
                for jo in range(JT):
                    nc.tensor.matmul(
                        ht[:, jo * BL:(jo + 1) * BL],
                        wg_blk(2, kc, jo), rh[:, kc * BL:(kc + 1) * BL],
                        start=False, stop=(kc == KC - 1 and jo == JT - 1),
                    )

            if pj is not None:
                proj_mms(t % PJ, pbase, (2, 3), pj)

            # ---- sigmoid(z); pre-products for the state update, in halves
            zf = workp.tile([128, FR], bf16, tag="zf")
            nc.scalar.activation(zf[:], zr[:, 0:FR], AF.Sigmoid, scale=1.0 / WSCALE)
            if not last:
                e2 = workp.tile([128, FR], bf16, tag="e2")
                q = workp.tile([128, FR], bf16, tag="q")
                u = workp.tile([128, FR], bf16, tag="u")
                for h0 in (0, HB):
                    sl = slice(h0, h0 + HB)
                    nc.vector.tensor_mul(e2[:, sl], gn[:, sl], zf[:, sl])
                    nc.vector.tensor_mul(q[:, sl], zf[:, sl], e1[:, sl])
                    nc.vector.tensor_sub(u[:, sl], e1[:, sl], q[:, sl])

                # next-step gate groups + u-pass of the pipelined r-gate
                zr_n = pzrp.tile([128, 2 * FR], f32, tag="zr")
                ht_n = phtp.tile([128, FR], f32, tag="ht")
                init_gates(zr_n, ht_n)
                rpass(zr_n, u, range(KC))

            # ---- tanh -> hd' = u + e2*htl in halves; v-pass follows each half
            htl = workp.tile([128, FR], bf16, tag="htl")
            if not last:
                v = workp.tile([128, FR], bf16, tag="v")
                hdb_n = statep.tile([128, FR], bf16, tag="hdb")
            for h0 in (0, HB):
                sl = slice(h0, h0 + HB)
                nc.scalar.activation(htl[:, sl], ht[:, sl], AF.Tanh,
                                     scale=1.0 / WSCALE)
                if not last:
                    nc.vector.tensor_mul(v[:, sl], e2[:, sl], htl[:, sl])
                    nc.vector.tensor_add(hdb_n[:, sl], u[:, sl], v[:, sl])
                    rpass(zr_n, v, (h0 // BL, h0 // BL + 1))

            # ---- h_t = hd + z*(htl-hd) on GPSIMD (slack-tolerant) -> history
            d1 = workp.tile([128, FR], bf16, tag="d1")
            nc.gpsimd.tensor_sub(d1[:], htl[:], hdb[:])
            d2 = workp.tile([128, FR], bf16, tag="d2")
            nc.gpsimd.tensor_mul(d2[:], zf[:], d1[:])
            nc.gpsimd.tensor_add(hs[:, t, :], hdb[:], d2[:])

            if pj is not None:
                proj_evac(t % PJ, pbase, pj)

            if not last:
                hdb = hdb_n
                zr, ht = zr_n, ht_n

        # ---- final projection flush for steps [T-PJ, T)
        for m in range(JT):
            pj = ppjp.tile([128, PJ * BL], f32, tag=f"pj{m}")
            proj_mms(m, T - PJ, range(KC), pj)
            proj_evac(m, T - PJ, pj)

    nc.compile()

    # LDW dedup surgery: consecutive identical weight loads (same AP, no sems)
    # collapse to one -- the PE array already holds that stationary operand.
    for blk in nc.main_func.blocks:
        keep = []
        prev_key = None
        for ins in blk.instructions:
            nm = type(ins).__name__
            if nm == 'InstLdweights':
                key = str(ins.ins[0])
                has_sem = ins.sync_info is not None and (
                    len(ins.sync_info.on_wait) > 0 or len(ins.sync_info.on_update) > 0)
                if key == prev_key and not has_sem:
                    continue
                prev_key = key
            elif nm in ('InstMatmult', 'InstEventSemaphore', 'InstDrain'):
                pass
            else:
                prev_key = None
            keep.append(ins)
        blk.instructions[:] = keep

    _BUILD_CACHE["nc"] = nc
    return nc


def _host_prep(C, t, Wz, bz, Wr, br, Wh, bh, Wgh, bgh, Wlin, blin):
    bf = ml_dtypes.bfloat16
    f8 = ml_dtypes.float8_e4m3

    s = Wgh.sum(axis=0)  # (H,)
    t3 = t[:, :, 0]  # (T,B)
    dt = np.concatenate([np.zeros((1, B), np.float32), t3[1:] - t3[:-1]], axis=0)
    gam = np.exp(-np.maximum(
        dt[:, :, None] * s[None, None, :] + bgh[None, None, :], 0.0)).astype(np.float32)

    def gate_const(W, b):
        return C @ W[0:H] + (W[2 * H:3 * H].sum(axis=0) + b)[None, :]

    Az0 = gate_const(Wz, bz) * WSCALE
    Ar0 = gate_const(Wr, br) * WSCALE
    Ah0 = gate_const(Wh, bh) * WSCALE

    # gate weight tiles, fp8 * WSCALE, packed [p, ((g*KC+kc)*JT+jo)*128 + col]
    Wg = np.stack([Wz[H:2 * H], Wr[H:2 * H], Wh[H:2 * H]])  # (3,H,H)
    wg8 = (Wg * WSCALE).reshape(3, KC, 128, JT, 128).transpose(2, 0, 1, 3, 4)
    wg8 = np.ascontiguousarray(wg8.reshape(128, 3 * KC * JT * 128), dtype=f8)
    # wlin tiles bf16, packed [p, (kc*JT+m)*128 + col]
    wl = Wlin.reshape(KC, 128, JT, 128).transpose(1, 0, 2, 3)
    wl = np.ascontiguousarray(wl.reshape(128, KC * JT * 128), dtype=bf)
    identv = np.ascontiguousarray(np.eye(128), dtype=f8)
    blinT = np.ascontiguousarray(
        blin.reshape(JT, 128).T, dtype=np.float32)  # [128, JT]

    in_maps = []
    for i in range(NCORES):
        sl = slice(i * BL, (i + 1) * BL)
        gf = gam[:, sl, :]  # (T,BL,H)
        gp = np.ascontiguousarray(
            gf.reshape(T, BL, KC, 128).transpose(3, 0, 2, 1).reshape(128, T, KC * BL),
            dtype=bf)

        def packA(A):
            return A[sl].reshape(BL, JT, 128).transpose(2, 1, 0).reshape(128, JT * BL)

        a0zr = np.ascontiguousarray(
            np.concatenate([packA(Az0), packA(Ar0)], axis=1), dtype=bf)
        in_maps.append({
            "gam": gp,
            "wg8": wg8,
            "wlin": wl,
            "a0zr": a0zr,
            "a0h": np.ascontiguousarray(packA(Ah0), dtype=bf),
            "ident": identv,
            "blin": blinT,
        })
    return in_maps


def kernel(C, t, mask, Wz, bz, Wr, br, Wh, bh, Wgh, bgh, wgx, bgx, Wlin, blin,
           _trace=False, _trace_kwargs=None):
    C = np.asarray(C, np.float32)
    t = np.asarray(t, np.float32)
    nc = _build_program()
    in_maps = _host_prep(C, t,
                         np.asarray(Wz, np.float32), np.asarray(bz, np.float32),
                         np.asarray(Wr, np.float32), np.asarray(br, np.float32),
                         np.asarray(Wh, np.float32), np.asarray(bh, np.float32),
                         np.asarray(Wgh, np.float32), np.asarray(bgh, np.float32),
                         np.asarray(Wlin, np.float32), np.asarray(blin, np.float32))

    from concourse.bass_utils import run_bass_kernel_spmd
    res = run_bass_kernel_spmd(nc, in_maps, list(range(NCORES)),
                               trace=_trace, **(_trace_kwargs or {}))
    outs = []
    for i in range(NCORES):
        o4 = np.asarray(res.results[i]["out"], dtype=np.float32)  # (JT,128,T,BL)
        outs.append(o4.transpose(2, 3, 0, 1).reshape(T, BL, O))
    full = np.concatenate(outs, axis=1)  # (T,B,O)
    kernel._last_results = res
    return full


# revision 8
# speedup vs baseline: 1.2208x; 1.2208x over previous
"""GRU-D decoder kernel for Trainium2 (8 NeuronCores, data-parallel over batch).

Math (mask == ones everywhere, which the reference hardcodes):
  x_hat = C (constant), d = dt broadcast, gamma_x unused.
  gamma[t,b,j] = exp(-relu(dt[t,b] * colsum(Wgh)[j] + bgh[j]))   (precomputed host-side)
  per step: hd = gamma_t * h
            z = sigmoid(hd @ Wz_h + Az0);  r = sigmoid(hd @ Wr_h + Ar0)
            htl = tanh((r*hd) @ Wh_h + Ah0)
            h = hd + z*(htl - hd)
  out[t] = h_t @ Wlin + blin
  where A?0 = C @ W?_x + colsum(W?_m) + b?  (time-constant, precomputed host-side).

Device design (v4):
  - Transposed world: H folded onto 128 partitions (4 chunks of 128), batch=64
    on the free axis.  Weight-stationary (form 2) gate matmuls, N=64 streams.
  - Gate weights quantized to fp8e4 * 256 (stationary; fast weight load), moving
    operand hd stays bf16; the 1/256 is folded into the activation scale.
  - State update with gamma folded in:
      hd' = u + v,  u = e1 - z*e1 (pre-tanh),  v = e2*tanh(...)
      e1 = gamma'*hd (GPSIMD, step start),  e2 = gamma'*z.
  - All element-wise work lives on DVE (GPSIMD turned out to slow concurrent
    DVE ops ~3x via SBUF port contention): pm = (z-1)*hd as one fused
    scalar_tensor_tensor, then after each tanh half: p2 = z*htl,
    h = p2 - pm (written straight into the h-history tile), hd' = gamma'*h.
  - z and r share one PSUM bank [128, 512]; one ident-matmul streams the
    (pre-scaled) gate constants to start both accumulation groups.
  - h_t = hd + z*(htl-hd) recomputed on GPSIMD (slack-tolerant) into a big
    h-history tile; output projection (form 2, bf16 Wlin) runs as a rolling
    4-step-batched matmul split around the candidate matmuls; +blin is fused
    into an ACT Identity-with-bias PSUM evacuation.  Output DMA'd as bf16.
"""

import numpy as np
import ml_dtypes

T, B, H, O = 100, 512, 512, 512
NCORES = 8
BL = B // NCORES  # 64
KC = 4            # contraction chunks of 128
JT = 4            # output j-tiles of 128
FR = JT * BL      # 256
HB = FR // 2      # 128 (half of the free dim)
GCH = 10          # gamma chunk (steps per DMA)
PJ = 4            # projection flush period (steps)
WSCALE = 256.0    # fp8 gate-weight scale (undone in the activation)

_BUILD_CACHE = {}


def _build_program():
    if "nc" in _BUILD_CACHE:
        return _BUILD_CACHE["nc"]

    import concourse.tile as tile
    import concourse.mybir as mybir
    from concourse import bacc
    from contextlib import ExitStack

    f32 = mybir.dt.float32
    bf16 = mybir.dt.bfloat16
    f8 = mybir.dt.float8e4
    AF = mybir.ActivationFunctionType
    ALU = mybir.AluOpType

    nc = bacc.Bacc("TRN2", target_bir_lowering=False, debug=False,
                   num_devices=NCORES)

    gam_d = nc.dram_tensor("gam", [128, T, FR], bf16, kind="ExternalInput")
    wg8_d = nc.dram_tensor("wg8", [128, 3 * KC * JT * 128], f8, kind="ExternalInput")
    wlin_d = nc.dram_tensor("wlin", [128, KC * JT * 128], bf16, kind="ExternalInput")
    a0zr_d = nc.dram_tensor("a0zr", [128, 2 * FR], bf16, kind="ExternalInput")
    a0h_d = nc.dram_tensor("a0h", [128, FR], bf16, kind="ExternalInput")
    ident_d = nc.dram_tensor("ident", [128, 128], f8, kind="ExternalInput")
    blin_d = nc.dram_tensor("blin", [128, JT], f32, kind="ExternalInput")
    out_d = nc.dram_tensor("out", [JT, 128, T, BL], bf16, kind="ExternalOutput")

    with tile.TileContext(nc) as tc, ExitStack() as ctx:
        constp = ctx.enter_context(tc.tile_pool(name="const", bufs=1))
        bigp = ctx.enter_context(tc.tile_pool(name="big", bufs=1))
        statep = ctx.enter_context(tc.tile_pool(name="state", bufs=2))
        workp = ctx.enter_context(tc.tile_pool(name="work", bufs=2))
        stagep = ctx.enter_context(tc.tile_pool(name="stage", bufs=3))
        pzrp = ctx.enter_context(tc.tile_pool(name="pzr", bufs=2, space="PSUM"))
        phtp = ctx.enter_context(tc.tile_pool(name="pht", bufs=2, space="PSUM"))
        ppjp = ctx.enter_context(tc.tile_pool(name="ppj", bufs=1, space="PSUM"))

        wg8 = constp.tile([128, 3 * KC * JT * 128], f8)
        nc.sync.dma_start(wg8[:], wg8_d[:])
        wlin = constp.tile([128, KC * JT * 128], bf16)
        nc.sync.dma_start(wlin[:], wlin_d[:])
        a0zr = constp.tile([128, 2 * FR], bf16)
        nc.sync.dma_start(a0zr[:], a0zr_d[:])
        a0h = constp.tile([128, FR], bf16)
        nc.sync.dma_start(a0h[:], a0h_d[:])
        ident = constp.tile([128, 128], f8)
        nc.sync.dma_start(ident[:], ident_d[:])
        blin = constp.tile([128, JT], f32)
        nc.sync.dma_start(blin[:], blin_d[:])

        # gamma history + h history as big 3D tiles; chunked DMA into slices
        gam = bigp.tile([128, T, FR], bf16)
        hs = bigp.tile([128, T, FR], bf16)

        def gam_fetch(c):
            t0 = c * GCH
            t1 = min(t0 + GCH, T)
            if t0 < T:
                nc.sync.dma_start(gam[:, t0:t1, :], gam_d[:, t0:t1, :])

        gam_fetch(0)
        gam_fetch(1)

        def wg_blk(g, kc, jo):
            i = ((g * KC + kc) * JT + jo) * 128
            return wg8[:, i:i + 128]

        def wl_blk(kc, m):
            i = (kc * JT + m) * 128
            return wlin[:, i:i + 128]

        hdb = statep.tile([128, FR], bf16, tag="hdb")
        nc.vector.memset(hdb[:], 0.0)

        def proj_mms(m, base, kcs, pj):
            for kc in kcs:
                nc.tensor.matmul(
                    pj[:, 0:PJ * BL],
                    wl_blk(kc, m),
                    hs[:, base:base + PJ, kc * BL:(kc + 1) * BL],
                    start=(kc == 0), stop=(kc == KC - 1),
                )

        def proj_evac(m, base, pj):
            stg = stagep.tile([128, PJ * BL], bf16, tag="stg")
            nc.scalar.activation(stg[:], pj[:], AF.Identity, bias=blin[:, m:m + 1])
            nc.sync.dma_start(out_d[m][:, base:base + PJ, :], stg[:])

        def init_gates(zr, ht):
            nc.tensor.matmul(zr[:], ident[:], a0zr[:], start=True, stop=False)
            nc.tensor.matmul(ht[:], ident[:], a0h[:], start=True, stop=False)

        def rpass(zr, src, kcs):
            # r-gate accumulation pass over contraction chunks kcs of src
            for kc in kcs:
                for jo in range(JT):
                    nc.tensor.matmul(
                        zr[:, FR + jo * BL:FR + (jo + 1) * BL],
                        wg_blk(1, kc, jo), src[:, kc * BL:(kc + 1) * BL],
                        start=False, stop=False,
                    )

        # step-0 gate PSUM groups (hd_0 = 0, so constants only for r)
        zr = pzrp.tile([128, 2 * FR], f32, tag="zr")
        ht = phtp.tile([128, FR], f32, tag="ht")
        init_gates(zr, ht)

        for t in range(T):
            c, o = divmod(t, GCH)
            if o == 0 and t > 0:
                gam_fetch(c + 1)
            last = t + 1 >= T

            if not last:
                gn = gam[:, t + 1, :]

            # ---- r-gate pass on hd (k-outer so it starts on the first
            # half of hd' from the previous step)
            rpass(zr, hdb, range(KC))

            # ---- sigmoid(r) -> rh in halves
            rb = workp.tile([128, FR], bf16, tag="rb")
            rh = workp.tile([128, FR], bf16, tag="rh")
            for h0 in (0, HB):
                nc.scalar.activation(rb[:, h0:h0 + HB], zr[:, FR + h0:FR + h0 + HB],
                                     AF.Sigmoid, scale=1.0 / WSCALE)
                nc.vector.tensor_mul(rh[:, h0:h0 + HB], rb[:, h0:h0 + HB],
                                     hdb[:, h0:h0 + HB])

            # ---- z-gate single pass on hd (runs during sigmoid(r) window)
            for kc in range(KC):
                for jo in range(JT):
                    nc.tensor.matmul(
                        zr[:, jo * BL:(jo + 1) * BL],
                        wg_blk(0, kc, jo), hdb[:, kc * BL:(kc + 1) * BL],
                        start=False, stop=(jo == JT - 1 and kc == KC - 1),
                    )
            _ = None

            # ---- rolling projection (PE filler) for steps [pbase, pbase+PJ)
            pj = pbase = None
            if t >= PJ:
                pbase = (t // PJ - 1) * PJ
                pj = ppjp.tile([128, PJ * BL], f32, tag=f"pj{t % PJ}")
                proj_mms(t % PJ, pbase, (0, 1), pj)

            # ---- candidate MMs (k-outer: kc chunk waits only on its rh half)
            for kc in range(KC):
                for jo in range(JT):
                    nc.tensor.matmul(
                        ht[:, jo * BL:(jo + 1) * BL],
                        wg_blk(2, kc, jo), rh[:, kc * BL:(kc + 1) * BL],
                        start=False, stop=(kc == KC - 1 and jo == JT - 1),
                    )

            if pj is not None:
                proj_mms(t % PJ, pbase, (2, 3), pj)

            # ---- sigmoid(z); pm = (z-1)*hd (one fused STT, off-path)
            zf = workp.tile([128, FR], bf16, tag="zf")
            nc.scalar.activation(zf[:], zr[:, 0:FR], AF.Sigmoid, scale=1.0 / WSCALE)
            pm = workp.tile([128, FR], bf16, tag="pm")
            nc.vector.scalar_tensor_tensor(pm[:], zf[:], 1.0, hdb[:],
                                           ALU.subtract, ALU.mult)

            if not last:
                # next-step gate PSUM groups (ident inits fill the tanh window)
                zr_n = pzrp.tile([128, 2 * FR], f32, tag="zr")
                ht_n = phtp.tile([128, FR], f32, tag="ht")
                init_gates(zr_n, ht_n)

            # ---- tanh -> h = z*htl - pm -> hd' = gamma'*h, in halves
            htl = workp.tile([128, FR], bf16, tag="htl")
            p2 = workp.tile([128, FR], bf16, tag="p2")
            if not last:
                hdb_n = statep.tile([128, FR], bf16, tag="hdb")
            for h0 in (0, HB):
                sl = slice(h0, h0 + HB)
                nc.scalar.activation(htl[:, sl], ht[:, sl], AF.Tanh,
                                     scale=1.0 / WSCALE)
                nc.vector.tensor_mul(p2[:, sl], zf[:, sl], htl[:, sl])
                nc.vector.tensor_sub(hs[:, t, sl], p2[:, sl], pm[:, sl])
                if not last:
                    nc.vector.tensor_mul(hdb_n[:, sl], gn[:, sl], hs[:, t, sl])

            if pj is not None:
                proj_evac(t % PJ, pbase, pj)

            if not last:
                hdb = hdb_n
                zr, ht = zr_n, ht_n

        # ---- final projection flush for steps [T-PJ, T)
        for m in range(JT):
            pj = ppjp.tile([128, PJ * BL], f32, tag=f"pj{m}")
            proj_mms(m, T - PJ, range(KC), pj)
            proj_evac(m, T - PJ, pj)

    nc.compile()

    # LDW dedup surgery: consecutive identical weight loads (same AP, no sems)
    # collapse to one -- the PE array already holds that stationary operand.
    for blk in nc.main_func.blocks:
        keep = []
        prev_key = None
        for ins in blk.instructions:
            nm = type(ins).__name__
            if nm == 'InstLdweights':
                key = str(ins.ins[0])
                has_sem = ins.sync_info is not None and (
                    len(ins.sync_info.on_wait) > 0 or len(ins.sync_info.on_update) > 0)
                if key == prev_key and not has_sem:
                    continue
                prev_key = key
            elif nm in ('InstMatmult', 'InstEventSemaphore', 'InstDrain'):
                pass
            else:
                prev_key = None
            keep.append(ins)
        blk.instructions[:] = keep

    _BUILD_CACHE["nc"] = nc
    return nc


def _host_prep(C, t, Wz, bz, Wr, br, Wh, bh, Wgh, bgh, Wlin, blin):
    bf = ml_dtypes.bfloat16
    f8 = ml_dtypes.float8_e4m3

    s = Wgh.sum(axis=0)  # (H,)
    t3 = t[:, :, 0]  # (T,B)
    dt = np.concatenate([np.zeros((1, B), np.float32), t3[1:] - t3[:-1]], axis=0)
    gam = np.exp(-np.maximum(
        dt[:, :, None] * s[None, None, :] + bgh[None, None, :], 0.0)).astype(np.float32)

    def gate_const(W, b):
        return C @ W[0:H] + (W[2 * H:3 * H].sum(axis=0) + b)[None, :]

    Az0 = gate_const(Wz, bz) * WSCALE
    Ar0 = gate_const(Wr, br) * WSCALE
    Ah0 = gate_const(Wh, bh) * WSCALE

    # gate weight tiles, fp8 * WSCALE, packed [p, ((g*KC+kc)*JT+jo)*128 + col]
    Wg = np.stack([Wz[H:2 * H], Wr[H:2 * H], Wh[H:2 * H]])  # (3,H,H)
    wg8 = (Wg * WSCALE).reshape(3, KC, 128, JT, 128).transpose(2, 0, 1, 3, 4)
    wg8 = np.ascontiguousarray(wg8.reshape(128, 3 * KC * JT * 128), dtype=f8)
    # wlin tiles bf16, packed [p, (kc*JT+m)*128 + col]
    wl = Wlin.reshape(KC, 128, JT, 128).transpose(1, 0, 2, 3)
    wl = np.ascontiguousarray(wl.reshape(128, KC * JT * 128), dtype=bf)
    identv = np.ascontiguousarray(np.eye(128), dtype=f8)
    blinT = np.ascontiguousarray(
        blin.reshape(JT, 128).T, dtype=np.float32)  # [128, JT]

    in_maps = []
    for i in range(NCORES):
        sl = slice(i * BL, (i + 1) * BL)
        gf = gam[:, sl, :]  # (T,BL,H)
        gp = np.ascontiguousarray(
            gf.reshape(T, BL, KC, 128).transpose(3, 0, 2, 1).reshape(128, T, KC * BL),
            dtype=bf)

        def packA(A):
            return A[sl].reshape(BL, JT, 128).transpose(2, 1, 0).reshape(128, JT * BL)

        a0zr = np.ascontiguousarray(
            np.concatenate([packA(Az0), packA(Ar0)], axis=1), dtype=bf)
        in_maps.append({
            "gam": gp,
            "wg8": wg8,
            "wlin": wl,
            "a0zr": a0zr,
            "a0h": np.ascontiguousarray(packA(Ah0), dtype=bf),
            "ident": identv,
            "blin": blinT,
        })
    return in_maps


def kernel(C, t, mask, Wz, bz, Wr, br, Wh, bh, Wgh, bgh, wgx, bgx, Wlin, blin,
           _trace=False, _trace_kwargs=None):
    C = np.asarray(C, np.float32)
    t = np.asarray(t, np.float32)
    nc = _build_program()
    in_maps = _host_prep(C, t,
                         np.asarray(Wz, np.float32), np.asarray(bz, np.float32),
                         np.asarray(Wr, np.float32), np.asarray(br, np.float32),
                         np.asarray(Wh, np.float32), np.asarray(bh, np.float32),
                         np.asarray(Wgh, np.float32), np.asarray(bgh, np.float32),
                         np.asarray(Wlin, np.float32), np.asarray(blin, np.float32))

    from concourse.bass_utils import run_bass_kernel_spmd
    res = run_bass_kernel_spmd(nc, in_maps, list(range(NCORES)),
                               trace=_trace, **(_trace_kwargs or {}))
    outs = []
    for i in range(NCORES):
        o4 = np.asarray(res.results[i]["out"], dtype=np.float32)  # (JT,128,T,BL)
        outs.append(o4.transpose(2, 3, 0, 1).reshape(T, BL, O))
    full = np.concatenate(outs, axis=1)  # (T,B,O)
    kernel._last_results = res
    return full
